# revision 10
# baseline (speedup 1.0000x reference)
"""Sliding-window multi-head attention on 8 TRN2 NeuronCores.

Sharding: data-parallel over batch (B=2) x tensor-parallel over head-pairs
(8 heads -> 4 pairs), one (batch, head-pair) per core. Each core computes
QKV projections for its 2 heads, banded attention (|i-j| <= 128) with
key-padding handled multiplicatively, and the per-head output projection
partials. Host divides by the softmax denominators, applies the query
padding mask, sums partials over head-pair shards and adds biases.

All matmuls run in bf16 with fp32 PSUM accumulation; the softmax
(exp / band mask / normalization) is carried in fp32/bf16.
"""

import numpy as np
import ml_dtypes

import concourse.bass as bass
import concourse.mybir as mybir
import concourse.tile as tile
from concourse import bass_utils

H = 8
Dh = 64
S = 2048
D = 512
B = 2
WIN = 256
HWIN = WIN // 2          # 128
NB = S // 128            # 16 s-blocks / k-chunks
NPAIR = NB // 2          # 8 chunk pairs
SEXT = S + 256           # Qt padded with one zero block each side
BF16 = mybir.dt.bfloat16
F32 = mybir.dt.float32

TRACE = {"on": False, "last": None}


def _split_excess_waits(nc):
    """This container's walrus accepts at most 1 sync-wait per instruction
    (2 on EventSemaphore); hoist excess waits onto same-engine NoOps."""
    for fn in nc.m.functions:
        for blk in fn.blocks:
            new_list, changed = [], False
            for inst in blk.instructions:
                si = inst.sync_info
                waits = list(si.on_wait) if si is not None and si.on_wait else []
                cap = 2 if isinstance(inst, mybir.InstEventSemaphore) else 1
                if len(waits) > cap:
                    changed = True
                    extra, keep = waits[:-cap], waits[-cap:]
                    for j, w in enumerate(extra):
                        new_list.append(mybir.InstNoOp(
                            name=f"{inst.name}-wsplit{j}", engine=inst.engine,
                            sync_info=mybir.SyncInfo(on_wait=[w], on_update=[])))
                    si.on_wait = keep
                    inst.sync_info = si
                new_list.append(inst)
            if changed:
                blk.instructions = new_list


def _build():
    nc = bass.Bass("TRN2", target_bir_lowering=False, debug=False)
    xt = nc.dram_tensor("xt", [4, 128, S], BF16, kind="ExternalInput")
    wqk = nc.dram_tensor("wqk", [2, 4, 128, 128], BF16, kind="ExternalInput")
    wv = nc.dram_tensor("wv", [4, 128, 128], BF16, kind="ExternalInput")
    wo = nc.dram_tensor("wo", [2, 64, 512], BF16, kind="ExternalInput")
    band2 = nc.dram_tensor("band2", [128, 768], BF16, kind="ExternalInput")
    padk = nc.dram_tensor("padk", [128, NB], F32, kind="ExternalInput")
    o_out = [nc.dram_tensor(f"o{h}", [NB, 128, 512], BF16, kind="ExternalOutput")
             for h in range(2)]
    dn = nc.dram_tensor("dn", [2, S], BF16, kind="ExternalOutput")

    with tile.TileContext(nc) as tc:
        with (
            tc.tile_pool(name="const", bufs=1) as cp,
            tc.tile_pool(name="big", bufs=1) as bp,
            tc.tile_pool(name="work", bufs=3) as wp,
        ):
            # ---- load inputs ----
            xts = []
            for d in range(4):
                xt_d = bp.tile([128, S], BF16, name=f"xt{d}")
                nc.sync.dma_start(out=xt_d, in_=xt[d])
                xts.append(xt_d)
            wqk_sb = []
            for h in range(2):
                row = []
                for d in range(4):
                    w_hd = cp.tile([128, 128], BF16, name=f"wqk{h}{d}")
                    nc.sync.dma_start(out=w_hd, in_=wqk[h, d])
                    row.append(w_hd)
                wqk_sb.append(row)
            wv_sb = []
            for d in range(4):
                wv_d = cp.tile([128, 128], BF16, name=f"wv{d}")
                nc.sync.dma_start(out=wv_d, in_=wv[d])
                wv_sb.append(wv_d)
            wo_sb = []
            for h in range(2):
                wo_h = cp.tile([64, 512], BF16, name=f"wo{h}")
                nc.sync.dma_start(out=wo_h, in_=wo[h])
                wo_sb.append(wo_h)
            band_sb = cp.tile([128, 768], BF16, name="bandsb")
            nc.sync.dma_start(out=band_sb, in_=band2[:, :])
            padk_sb = cp.tile([128, NB], F32, name="padksb")
            nc.sync.dma_start(out=padk_sb, in_=padk[:, :])

            psA_ctx = tc.tile_pool(name="psA", bufs=2, space="PSUM")
            psA = psA_ctx.__enter__()
            # ---- QK projection ----
            # qt_sb rows 0:64 = head0 Qt, rows 64:128 = head1 Qt (ext-padded);
            # kt_sb likewise for Kt. wqk[0] = Wq both heads, wqk[1] = Wk.
            qt_sb = bp.tile([128, SEXT], BF16, name="qtsb")
            kt_sb = bp.tile([128, S], BF16, name="ktsb")
            nc.gpsimd.memset(qt_sb[:, 0:128], 0.0)
            nc.gpsimd.memset(qt_sb[:, S + 128:SEXT], 0.0)
            for g in range(2):  # 0 = q, 1 = k
                for t in range(4):
                    ps = psA.tile([128, 512], F32, tag="qkps", name=f"qkps{g}{t}")
                    for d in range(4):
                        nc.tensor.matmul(ps, wqk_sb[g][d],
                                         xts[d][:, 512 * t:512 * (t + 1)],
                                         start=(d == 0), stop=(d == 3))
                    if g == 0:
                        nc.vector.tensor_copy(
                            out=qt_sb[:, 128 + 512 * t:128 + 512 * (t + 1)], in_=ps)
                    else:
                        nc.vector.tensor_copy(
                            out=kt_sb[:, 512 * t:512 * (t + 1)], in_=ps)

            # ---- V projection (natural layout, per k-chunk), key-pad masked ----
            # v_sb[j] cols: [v_h0 (0:64) | padk (64) | v_h1 (65:129) | padk (129)]
            v_sb = []
            for j in range(NB):
                v_j = bp.tile([128, 130], BF16, name=f"v{j}")
                v_sb.append(v_j)
            for j in range(NB):
                ps = psA.tile([128, 128], F32, tag="vps", bufs=4, name=f"vps{j}")
                for d in range(4):
                    nc.tensor.matmul(ps, xts[d][:, 128 * j:128 * (j + 1)],
                                     wv_sb[d], start=(d == 0), stop=(d == 3))
                dst = v_sb[j][:, 0:130].rearrange("p (g c) -> p g c", g=2)[:, :, 0:64]
                src = ps[:, 0:128].rearrange("p (g c) -> p g c", g=2)
                nc.vector.tensor_scalar_mul(out=dst, in0=src,
                                            scalar1=padk_sb[:, j:j + 1])
                nc.gpsimd.tensor_copy(out=v_sb[j][:, 64:65],
                                      in_=padk_sb[:, j:j + 1])
                nc.gpsimd.tensor_copy(out=v_sb[j][:, 129:130],
                                      in_=padk_sb[:, j:j + 1])

            psA_ctx.__exit__(None, None, None)
            psS_ctx = tc.tile_pool(name="psS", bufs=3, space="PSUM")
            psS = psS_ctx.__enter__()
            psY_ctx = tc.tile_pool(name="psY", bufs=1, space="PSUM")
            psY = psY_ctx.__enter__()
            psO_ctx = tc.tile_pool(name="psO", bufs=1, space="PSUM")
            psO = psO_ctx.__enter__()
            # ---- attention per head ----
            yt_sb = []
            for h in range(2):
                yt_ps = psY.tile([65, S], F32, tag="ytps", name=f"ytps{h}")
                ptms = {}

                def do_block(i, h=h, yt_ps=yt_ps, ptms=ptms):
                    for j in range(max(0, i - 1), min(NB, i + 2)):
                        sub = 128 * (i - j + 1)
                        nc.tensor.matmul(
                            yt_ps[:, 128 * i:128 * (i + 1)],
                            v_sb[j][:, 65 * h:65 * h + 65],
                            ptms[j][:, sub:sub + 128],
                            start=(j == max(0, i - 1)),
                            stop=(j == min(NB - 1, i + 1)))

                for j in range(NB):
                    sps = psS.tile([128, 384], F32, tag="sps", name=f"sps{h}{j}")
                    nc.tensor.matmul(
                        sps,
                        kt_sb[64 * h:64 * h + 64, 128 * j:128 * (j + 1)],
                        qt_sb[64 * h:64 * h + 64, 128 * j:128 * j + 384],
                        start=True, stop=True)
                    pt = wp.tile([128, 384], BF16, tag="pt", bufs=4, name=f"pt{h}{j}")
                    nc.scalar.activation(out=pt, in_=sps,
                                         func=mybir.ActivationFunctionType.Exp,
                                         scale=0.125)
                    ptm = wp.tile([128, 384], BF16, tag="ptm", bufs=4,
                                  name=f"ptm{h}{j}")
                    if j % 2 == 0:
                        nc.gpsimd.tensor_mul(out=ptm, in0=pt, in1=band_sb[:, 0:384])
                    else:
                        nc.vector.tensor_mul(out=ptm, in0=pt, in1=band_sb[:, 0:384])
                    ptms[j] = ptm
                    if j >= 1:
                        do_block(j - 1)
                do_block(NB - 1)
                y_h = bp.tile([65, S], BF16, name=f"yt{h}")
                nc.vector.tensor_copy(out=y_h, in_=yt_ps)
                nc.sync.dma_start(out=dn[h:h + 1, :], in_=y_h[64:65, :])
                yt_sb.append(y_h)
                for i in range(NB):
                    ops = psO.tile([128, 512], F32, tag="ops", name=f"ops{h}{i}")
                    nc.tensor.matmul(ops, y_h[0:64, 128 * i:128 * (i + 1)],
                                     wo_sb[h], start=True, stop=True)
                    o_t = wp.tile([128, 512], BF16, tag="ot", bufs=4,
                                  name=f"ot{h}{i}")
                    if (h * NB + i) % 2 == 0:
                        nc.vector.tensor_copy(out=o_t, in_=ops)
                    else:
                        nc.scalar.copy(out=o_t, in_=ops)
                    nc.sync.dma_start(out=o_out[h][i], in_=o_t)
            psO_ctx.__exit__(None, None, None)
            psY_ctx.__exit__(None, None, None)
            psS_ctx.__exit__(None, None, None)

    _split_excess_waits(nc)
    return nc


_NC = None


def kernel(x, padding_mask, Wqkv, bqkv, Wo, bo):
    global _NC
    x = np.asarray(x, dtype=np.float32)
    padding_mask = np.asarray(padding_mask)
    Wqkv = np.asarray(Wqkv, dtype=np.float32)
    bqkv = np.asarray(bqkv, dtype=np.float32)
    Wo = np.asarray(Wo, dtype=np.float32)
    bo = np.asarray(bo, dtype=np.float32)

    if _NC is None:
        _NC = _build()
    nc = _NC

    # band2[p, c'] over a 384-wide q-window at k-chunk offset: valid iff
    # 0 <= (c - p) <= 256 with c = c' % 384 (tile doubled for paired chunks)
    p_i = np.arange(128)[:, None]
    c_i = np.arange(384)[None, :]
    band = ((c_i - p_i >= 0) & (c_i - p_i <= WIN)).astype(np.float32)
    band2 = np.concatenate([band, band], axis=1).astype(ml_dtypes.bfloat16)

    in_maps = []
    for c in range(8):
        b, hp = c // 4, c % 4
        h0 = 2 * hp
        xtb = np.ascontiguousarray(x[b].T).reshape(4, 128, S)
        wqk_c = np.stack([
            np.ascontiguousarray(np.concatenate(
                [Wqkv[192 * (h0 + hh) + 64 * g:192 * (h0 + hh) + 64 * (g + 1), :]
                 for hh in range(2)], axis=0).T).reshape(4, 128, 128)
            for g in range(2)])  # g=0: q both heads, g=1: k both heads
        wv_c = np.ascontiguousarray(np.concatenate(
            [Wqkv[192 * (h0 + hh) + 128:192 * (h0 + hh) + 192, :] for hh in range(2)],
            axis=0).T).reshape(4, 128, 128)
        wo_c = np.stack([
            np.ascontiguousarray(Wo[:, 64 * (h0 + hh):64 * (h0 + hh + 1)].T)
            for hh in range(2)])
        padk_c = np.ascontiguousarray(
            padding_mask[b].astype(np.float32).reshape(NB, 128).T)
        in_maps.append({
            "xt": xtb.astype(ml_dtypes.bfloat16),
            "wqk": wqk_c.astype(ml_dtypes.bfloat16),
            "wv": wv_c.astype(ml_dtypes.bfloat16),
            "wo": wo_c.astype(ml_dtypes.bfloat16),
            "band2": band2,
            "padk": padk_c,
        })

    res = bass_utils.run_bass_kernel_spmd(
        nc, in_maps, core_ids=list(range(8)), trace=TRACE["on"])
    TRACE["last"] = res

    out = np.zeros((B, S, D), dtype=np.float32)
    padf = padding_mask.astype(np.float32)
    for c in range(8):
        b = c // 4
        r = res.results[c]
        dnv = r["dn"].astype(np.float32)          # [2, S]
        for hh in range(2):
            o_h = r[f"o{hh}"].astype(np.float32).reshape(S, D)
            scale = padf[b] / np.maximum(dnv[hh], 1e-30)
            out[b] += o_h * scale[:, None]

    # biases: bo always; attention-side v-bias correction (exact since
    # softmax rows sum to 1 on valid rows): O += pad * (bv_h @ Wo_h^T)
    if np.any(bqkv):
        bv = bqkv.reshape(H, 3, Dh)[:, 2, :]          # [H, Dh]
        bq = bqkv.reshape(H, 3, Dh)[:, 0, :]
        bk = bqkv.reshape(H, 3, Dh)[:, 1, :]
        if np.any(bq) or np.any(bk):
            raise NotImplementedError("nonzero q/k bias not supported")
        corr = np.einsum("hd,ohd->o", bv, Wo.reshape(D, H, Dh))
        out += padf[:, :, None] * corr[None, None, :]
    out += bo[None, None, :]
    return out


# revision 11
# speedup vs baseline: 1.0075x; 1.0075x over previous
"""Sliding-window multi-head attention on 8 TRN2 NeuronCores.

Sharding: data-parallel over batch (B=2) x tensor-parallel over head-pairs
(8 heads -> 4 pairs), one (batch, head-pair) per core. Each core computes
QKV projections for its 2 heads, banded attention (|i-j| <= 128) with
key-padding handled multiplicatively, and the per-head output projection
partials. Host divides by the softmax denominators, applies the query
padding mask, sums partials over head-pair shards and adds biases.

All matmuls run in bf16 with fp32 PSUM accumulation; the softmax
(exp / band mask / normalization) is carried in fp32/bf16.
"""

import numpy as np
import ml_dtypes

import concourse.bass as bass
import concourse.mybir as mybir
import concourse.tile as tile
from concourse import bass_utils

H = 8
Dh = 64
S = 2048
D = 512
B = 2
WIN = 256
HWIN = WIN // 2          # 128
NB = S // 128            # 16 s-blocks / k-chunks
NPAIR = NB // 2          # 8 chunk pairs
SEXT = S + 256           # Qt padded with one zero block each side
BF16 = mybir.dt.bfloat16
F32 = mybir.dt.float32

TRACE = {"on": False, "last": None}


def _split_excess_waits(nc):
    """This container's walrus accepts at most 1 sync-wait per instruction
    (2 on EventSemaphore); hoist excess waits onto same-engine NoOps."""
    for fn in nc.m.functions:
        for blk in fn.blocks:
            new_list, changed = [], False
            for inst in blk.instructions:
                si = inst.sync_info
                waits = list(si.on_wait) if si is not None and si.on_wait else []
                cap = 2 if isinstance(inst, mybir.InstEventSemaphore) else 1
                if len(waits) > cap:
                    changed = True
                    extra, keep = waits[:-cap], waits[-cap:]
                    for j, w in enumerate(extra):
                        new_list.append(mybir.InstNoOp(
                            name=f"{inst.name}-wsplit{j}", engine=inst.engine,
                            sync_info=mybir.SyncInfo(on_wait=[w], on_update=[])))
                    si.on_wait = keep
                    inst.sync_info = si
                new_list.append(inst)
            if changed:
                blk.instructions = new_list


def _build():
    nc = bass.Bass("TRN2", target_bir_lowering=False, debug=False)
    xt = nc.dram_tensor("xt", [4, 128, S], BF16, kind="ExternalInput")
    wqk = nc.dram_tensor("wqk", [2, 4, 128, 128], BF16, kind="ExternalInput")
    wv = nc.dram_tensor("wv", [4, 128, 128], BF16, kind="ExternalInput")
    wo = nc.dram_tensor("wo", [2, 64, 512], BF16, kind="ExternalInput")
    band2 = nc.dram_tensor("band2", [128, 768], BF16, kind="ExternalInput")
    padk = nc.dram_tensor("padk", [128, NB], F32, kind="ExternalInput")
    o_out = [nc.dram_tensor(f"o{h}", [NB, 128, 512], BF16, kind="ExternalOutput")
             for h in range(2)]
    dn = nc.dram_tensor("dn", [2, S], BF16, kind="ExternalOutput")

    with tile.TileContext(nc) as tc:
        with (
            tc.tile_pool(name="const", bufs=1) as cp,
            tc.tile_pool(name="big", bufs=1) as bp,
            tc.tile_pool(name="work", bufs=3) as wp,
        ):
            # ---- load inputs ----
            xts = []
            for d in range(4):
                xt_d = bp.tile([128, S], BF16, name=f"xt{d}")
                nc.sync.dma_start(out=xt_d, in_=xt[d])
                xts.append(xt_d)
            wqk_sb = []
            for h in range(2):
                row = []
                for d in range(4):
                    w_hd = cp.tile([128, 128], BF16, name=f"wqk{h}{d}")
                    nc.sync.dma_start(out=w_hd, in_=wqk[h, d])
                    row.append(w_hd)
                wqk_sb.append(row)
            wv_sb = []
            for d in range(4):
                wv_d = cp.tile([128, 128], BF16, name=f"wv{d}")
                nc.sync.dma_start(out=wv_d, in_=wv[d])
                wv_sb.append(wv_d)
            wo_sb = []
            for h in range(2):
                wo_h = cp.tile([64, 512], BF16, name=f"wo{h}")
                nc.sync.dma_start(out=wo_h, in_=wo[h])
                wo_sb.append(wo_h)
            band_sb = cp.tile([128, 768], BF16, name="bandsb")
            nc.sync.dma_start(out=band_sb, in_=band2[:, :])
            padk_sb = cp.tile([128, NB], F32, name="padksb")
            nc.sync.dma_start(out=padk_sb, in_=padk[:, :])

            psA_ctx = tc.tile_pool(name="psA", bufs=2, space="PSUM")
            psA = psA_ctx.__enter__()
            # ---- QK projection ----
            # qt_sb rows 0:64 = head0 Qt, rows 64:128 = head1 Qt (ext-padded);
            # kt_sb likewise for Kt. wqk[0] = Wq both heads, wqk[1] = Wk.
            qt_sb = bp.tile([128, SEXT], BF16, name="qtsb")
            kt_sb = bp.tile([128, S], BF16, name="ktsb")
            nc.gpsimd.memset(qt_sb[:, 0:128], 0.0)
            nc.gpsimd.memset(qt_sb[:, S + 128:SEXT], 0.0)
            for g in range(2):  # 0 = q, 1 = k
                for t in range(4):
                    ps = psA.tile([128, 512], F32, tag="qkps", name=f"qkps{g}{t}")
                    for d in range(4):
                        nc.tensor.matmul(ps, wqk_sb[g][d],
                                         xts[d][:, 512 * t:512 * (t + 1)],
                                         start=(d == 0), stop=(d == 3))
                    if g == 0:
                        nc.vector.tensor_copy(
                            out=qt_sb[:, 128 + 512 * t:128 + 512 * (t + 1)], in_=ps)
                    else:
                        nc.vector.tensor_copy(
                            out=kt_sb[:, 512 * t:512 * (t + 1)], in_=ps)

            # ---- V projection (natural layout, per k-chunk), key-pad masked ----
            # v_sb[j] cols: [v_h0 (0:64) | padk (64) | v_h1 (65:129) | padk (129)]
            v_sb = []
            for j in range(NB):
                v_j = bp.tile([128, 130], BF16, name=f"v{j}")
                v_sb.append(v_j)
            for j in range(NB):
                ps = psA.tile([128, 128], F32, tag="vps", bufs=4, name=f"vps{j}")
                for d in range(4):
                    nc.tensor.matmul(ps, xts[d][:, 128 * j:128 * (j + 1)],
                                     wv_sb[d], start=(d == 0), stop=(d == 3))
                dst = v_sb[j][:, 0:130].rearrange("p (g c) -> p g c", g=2)[:, :, 0:64]
                src = ps[:, 0:128].rearrange("p (g c) -> p g c", g=2)
                nc.vector.tensor_scalar_mul(out=dst, in0=src,
                                            scalar1=padk_sb[:, j:j + 1])
                nc.gpsimd.tensor_copy(out=v_sb[j][:, 64:65],
                                      in_=padk_sb[:, j:j + 1])
                nc.gpsimd.tensor_copy(out=v_sb[j][:, 129:130],
                                      in_=padk_sb[:, j:j + 1])

            psA_ctx.__exit__(None, None, None)
            psS_ctx = tc.tile_pool(name="psS", bufs=4, space="PSUM")
            psS = psS_ctx.__enter__()
            psY_ctx = tc.tile_pool(name="psY", bufs=1, space="PSUM")
            psY = psY_ctx.__enter__()
            # ---- attention per head ----
            yt_sb = []
            for h in range(2):
                yt_ps = psY.tile([65, S], F32, tag="ytps", name=f"ytps{h}")
                ptms = {}

                def do_block(i, h=h, yt_ps=yt_ps, ptms=ptms):
                    for j in range(max(0, i - 1), min(NB, i + 2)):
                        sub = 128 * (i - j + 1)
                        nc.tensor.matmul(
                            yt_ps[:, 128 * i:128 * (i + 1)],
                            v_sb[j][:, 65 * h:65 * h + 65],
                            ptms[j][:, sub:sub + 128],
                            start=(j == max(0, i - 1)),
                            stop=(j == min(NB - 1, i + 1)))

                for j in range(NB):
                    sps = psS.tile([128, 384], F32, tag="sps", name=f"sps{h}{j}")
                    nc.tensor.matmul(
                        sps,
                        kt_sb[64 * h:64 * h + 64, 128 * j:128 * (j + 1)],
                        qt_sb[64 * h:64 * h + 64, 128 * j:128 * j + 384],
                        start=True, stop=True)
                    pt = wp.tile([128, 384], BF16, tag="pt", bufs=4, name=f"pt{h}{j}")
                    nc.scalar.activation(out=pt, in_=sps,
                                         func=mybir.ActivationFunctionType.Exp,
                                         scale=0.125)
                    ptm = wp.tile([128, 384], BF16, tag="ptm", bufs=4,
                                  name=f"ptm{h}{j}")
                    if j % 2 == 0:
                        nc.gpsimd.tensor_mul(out=ptm, in0=pt, in1=band_sb[:, 0:384])
                    else:
                        nc.vector.tensor_mul(out=ptm, in0=pt, in1=band_sb[:, 0:384])
                    ptms[j] = ptm
                    if j >= 1:
                        do_block(j - 1)
                do_block(NB - 1)
                y_h = bp.tile([65, S], BF16, name=f"yt{h}")
                nc.vector.tensor_copy(out=y_h, in_=yt_ps)
                nc.sync.dma_start(out=dn[h:h + 1, :], in_=y_h[64:65, :])
                yt_sb.append(y_h)
            psY_ctx.__exit__(None, None, None)
            psS_ctx.__exit__(None, None, None)
            psO_ctx = tc.tile_pool(name="psO", bufs=2, space="PSUM")
            psO = psO_ctx.__enter__()
            # ---- per-head output projection partials ----
            for h in range(2):
                for i in range(NB):
                    ops = psO.tile([128, 512], F32, tag="ops", name=f"ops{h}{i}")
                    nc.tensor.matmul(ops, yt_sb[h][0:64, 128 * i:128 * (i + 1)],
                                     wo_sb[h], start=True, stop=True)
                    o_t = wp.tile([128, 512], BF16, tag="ot", bufs=4, name=f"ot{h}{i}")
                    if (h * NB + i) % 2 == 0:
                        nc.vector.tensor_copy(out=o_t, in_=ops)
                    else:
                        nc.scalar.copy(out=o_t, in_=ops)
                    nc.sync.dma_start(out=o_out[h][i], in_=o_t)
            psO_ctx.__exit__(None, None, None)

    _split_excess_waits(nc)
    return nc


_NC = None


def kernel(x, padding_mask, Wqkv, bqkv, Wo, bo):
    global _NC
    x = np.asarray(x, dtype=np.float32)
    padding_mask = np.asarray(padding_mask)
    Wqkv = np.asarray(Wqkv, dtype=np.float32)
    bqkv = np.asarray(bqkv, dtype=np.float32)
    Wo = np.asarray(Wo, dtype=np.float32)
    bo = np.asarray(bo, dtype=np.float32)

    if _NC is None:
        _NC = _build()
    nc = _NC

    # band2[p, c'] over a 384-wide q-window at k-chunk offset: valid iff
    # 0 <= (c - p) <= 256 with c = c' % 384 (tile doubled for paired chunks)
    p_i = np.arange(128)[:, None]
    c_i = np.arange(384)[None, :]
    band = ((c_i - p_i >= 0) & (c_i - p_i <= WIN)).astype(np.float32)
    band2 = np.concatenate([band, band], axis=1).astype(ml_dtypes.bfloat16)

    in_maps = []
    for c in range(8):
        b, hp = c // 4, c % 4
        h0 = 2 * hp
        xtb = np.ascontiguousarray(x[b].T).reshape(4, 128, S)
        wqk_c = np.stack([
            np.ascontiguousarray(np.concatenate(
                [Wqkv[192 * (h0 + hh) + 64 * g:192 * (h0 + hh) + 64 * (g + 1), :]
                 for hh in range(2)], axis=0).T).reshape(4, 128, 128)
            for g in range(2)])  # g=0: q both heads, g=1: k both heads
        wv_c = np.ascontiguousarray(np.concatenate(
            [Wqkv[192 * (h0 + hh) + 128:192 * (h0 + hh) + 192, :] for hh in range(2)],
            axis=0).T).reshape(4, 128, 128)
        wo_c = np.stack([
            np.ascontiguousarray(Wo[:, 64 * (h0 + hh):64 * (h0 + hh + 1)].T)
            for hh in range(2)])
        padk_c = np.ascontiguousarray(
            padding_mask[b].astype(np.float32).reshape(NB, 128).T)
        in_maps.append({
            "xt": xtb.astype(ml_dtypes.bfloat16),
            "wqk": wqk_c.astype(ml_dtypes.bfloat16),
            "wv": wv_c.astype(ml_dtypes.bfloat16),
            "wo": wo_c.astype(ml_dtypes.bfloat16),
            "band2": band2,
            "padk": padk_c,
        })

    res = bass_utils.run_bass_kernel_spmd(
        nc, in_maps, core_ids=list(range(8)), trace=TRACE["on"])
    TRACE["last"] = res

    out = np.zeros((B, S, D), dtype=np.float32)
    padf = padding_mask.astype(np.float32)
    for c in range(8):
        b = c // 4
        r = res.results[c]
        dnv = r["dn"].astype(np.float32)          # [2, S]
        for hh in range(2):
            o_h = r[f"o{hh}"].astype(np.float32).reshape(S, D)
            scale = padf[b] / np.maximum(dnv[hh], 1e-30)
            out[b] += o_h * scale[:, None]

    # biases: bo always; attention-side v-bias correction (exact since
    # softmax rows sum to 1 on valid rows): O += pad * (bv_h @ Wo_h^T)
    if np.any(bqkv):
        bv = bqkv.reshape(H, 3, Dh)[:, 2, :]          # [H, Dh]
        bq = bqkv.reshape(H, 3, Dh)[:, 0, :]
        bk = bqkv.reshape(H, 3, Dh)[:, 1, :]
        if np.any(bq) or np.any(bk):
            raise NotImplementedError("nonzero q/k bias not supported")
        corr = np.einsum("hd,ohd->o", bv, Wo.reshape(D, H, Dh))
        out += padf[:, :, None] * corr[None, None, :]
    out += bo[None, None, :]
    return out
